# revision 22
# baseline (speedup 1.0000x reference)
"""Trainium2 Bass kernel for nn_PixelBlock (causal-conv GLU backbone + causal
self-attention block). Data-parallel over batch: 8 images -> 8 NeuronCores.

Layout notes (per core, B=1):
- Activations are channel-major [C, S] with C on partitions (256 -> 2 tiles).
- For the 2x3 causal convs, spatial is kept in a padded layout: 33 rows
  (1 top-pad) x 34 cols (2 left-pad) = 1122 flat positions. Each conv tap is
  then a pure flat shift, so conv = 6 accumulated matmuls per K-tile.
- ELU(x) is computed as max(x,0) + min(exp(x),1) - 1. We carry h' = elu+1
  (so padding cells hold exactly 1.0) and fold the -1 into the next layer's
  bias via  b_eff = b - W @ ones  (computed on host).
- Sigmoid is computed via tanh (same ACT table set as exp -> no table swaps).
- Attention: scores are built transposed (sT[k,q] = k_h^T q_h), exp'd with
  the 1/sqrt(dk) scale folded into the ACT scale, diagonal 128-blocks masked
  with a 0/1 lower-triangle mask. PV uses V^T (computed directly by a
  transposed projection) augmented with a ones-column so the softmax
  denominators fall out of the same matmul. Output is normalized by a
  reciprocal broadcast through a tiny K=8 indicator matmul.
"""

import sys
import numpy as np
import ml_dtypes

sys.path.insert(0, "/opt/trn_rl_repo")

from contextlib import ExitStack

import concourse.bass as bass
import concourse.tile as tile
from concourse import bacc, mybir
from concourse.bass_utils import run_bass_kernel_spmd

F32 = mybir.dt.float32
FR = mybir.dt.float32r
BF = mybir.dt.bfloat16
ALU = mybir.AluOpType
ACTF = mybir.ActivationFunctionType

C = 256
S = 1024
NH = 8
DK = 32
PW = 34          # padded row width (2 left pad + 32)
NROW = 33        # 1 top pad + 32 rows
PTOT = NROW * PW  # 1122
P0 = 36          # first valid output position (row 1, col 2)
CHUNK = 362      # conv output chunk (3 * 362 = 1086 = PTOT - P0)
CHUNKS = [(P0 + i * CHUNK, P0 + (i + 1) * CHUNK) for i in range(3)]
# tap shifts for (di, dj) in row-major order, shift = (di-1)*34 + (dj-2)
SHIFTS = [-36, -35, -34, -2, -1, 0]
INV_SQRT_DK = 1.0 / np.sqrt(32.0)


def _build_program():
    nc = bacc.Bacc("TRN2", target_bir_lowering=False, debug=False, num_devices=8)

    d = {}
    def din(name, shape, dt=F32):
        d[name] = nc.dram_tensor(name, list(shape), dt, kind="ExternalInput").ap()
    din("inp", (C, S))
    din("c1w", (2, 2, 128, 6 * C), FR)   # [r, it, p, tap*o] packed
    din("c2w", (2, 2, 128, 12 * C), BF)  # [r, it, p, tap*o] packed
    din("b1e", (2, C))              # rb_b_in - sum(w)
    din("b2a", (2, C))              # a-part bias (already * 1.0)
    din("b2gh", (2, C))             # 0.5 * g-part bias
    din("apw", (2, 128, 5 * C), FR)       # [it, p, {pwin,wq,wk,wv,pwout}*o]
    din("pwinb", (C,))
    din("bqe", (C,))
    din("bke", (C,))
    din("s2row", (1, C), FR)        # wv.sum(1) - bv
    din("pwoutb", (C,))
    out_d = nc.dram_tensor("out", [C, S], F32, kind="ExternalOutput").ap()

    with tile.TileContext(nc) as tc, ExitStack() as ctx:
        _kernel_body(ctx, tc, nc, d, out_d)

    nc.compile()
    return nc


def _load_bias_col(nc, pool, dram_ap_1d, tag):
    """[256] dram vector -> two [128,1] sbuf column tiles."""
    tiles = []
    for t in range(2):
        b = pool.tile([128, 1], F32, tag=f"{tag}{t}")
        nc.sync.dma_start(out=b[:], in_=dram_ap_1d[t * 128:(t + 1) * 128].unsqueeze(1))
        tiles.append(b)
    return tiles


def _kernel_body(ctx, tc, nc, d, out_d):
    consts = ctx.enter_context(tc.tile_pool(name="consts", bufs=1))
    wq_pool = ctx.enter_context(tc.tile_pool(name="wq", bufs=1))

    # ---- constants (gpsimd-built masks are emitted in the attention
    # section: gpsimd takes ~26us to boot and must stay off the critical
    # path of the backbone's DVE stream) ----
    ones_f = consts.tile([1, 128], F32)
    nc.vector.memset(ones_f[:], 1.0)
    ones_row = consts.tile([1, 128], FR)
    nc.vector.tensor_copy(ones_row[:], ones_f[:])
    ones8 = consts.tile([128, 8], F32)
    nc.vector.memset(ones8[:], 1.0)
    ones64 = consts.tile([128, 64], F32)
    nc.vector.memset(ones64[:], 1.0)

    def build_masks():
        causal = consts.tile([128, 128], F32, name="causal")   # 1 where k <= q
        nc.gpsimd.memset(causal[:], 1.0)
        nc.gpsimd.affine_select(
            out=causal[:], in_=causal[:], compare_op=ALU.is_ge, fill=0.0,
            base=0, channel_multiplier=-1, pattern=[[1, 128]])
        ind4_f = consts.tile([4, 128], F32, name="ind4_f")     # 1 where m//32 == k
        nc.gpsimd.memset(ind4_f[:], 1.0)
        nc.gpsimd.affine_select(
            out=ind4_f[:], in_=ind4_f[:], compare_op=ALU.is_ge, fill=0.0,
            base=0, channel_multiplier=-32, pattern=[[1, 128]])
        nc.gpsimd.affine_select(
            out=ind4_f[:], in_=ind4_f[:], compare_op=ALU.is_ge, fill=0.0,
            base=31, channel_multiplier=32, pattern=[[-1, 128]])
        ind4 = consts.tile([4, 128], FR, name="ind4")
        nc.vector.tensor_copy(ind4[:], ind4_f[:])
        return causal, ind4


    # ---- attention / pointwise weights (packed, persistent) ----
    def load_apw():
        groups = [[], [], [], [], []]
        for t in range(2):
            w = wq_pool.tile([128, 5 * C], FR, tag=f"apw{t}", name=f"apw{t}")
            nc.sync.dma_start(out=w[:], in_=d["apw"][t])
            for g in range(5):
                groups[g].append(w[:, g * C:(g + 1) * C])
        return groups

    # ---- input -> padded layout ----
    def pad3(t2d):
        return t2d.rearrange("p (r c) -> p r c", r=NROW, c=PW)

    bb_stack = ExitStack()
    bb = bb_stack.enter_context(tc.tile_pool(name="bb", bufs=1))
    x_cur = []
    for t in range(2):
        x0 = bb.tile([128, PTOT], F32, name=f"x0_{t}", tag=f"xp{t}", bufs=2)
        nc.vector.memset(x0[:, 0:P0], 0.0)
        nc.vector.memset(pad3(x0)[:, 2:33, 0:2], 0.0)
        for hf in range(2):
            stg = bb.tile([128, 512], F32, name=f"stg{t}{hf}", tag="stg", bufs=4)
            nc.sync.dma_start(out=stg[:], in_=d["inp"][t * 128:(t + 1) * 128, hf * 512:(hf + 1) * 512])
            nc.vector.tensor_copy(
                pad3(x0)[:, 1 + 16 * hf:17 + 16 * hf, 2:34],
                stg[:].rearrange("p (r c) -> p r c", r=16, c=32))
        x_cur.append(x0)

    # ---- biases (emitted after input/weight DMAs; needed post-conv1) ----
    b1 = [_load_bias_col(nc, consts, d["b1e"][r], f"b1_{r}") for r in range(2)]
    b2a = [_load_bias_col(nc, consts, d["b2a"][r], f"b2a_{r}") for r in range(2)]
    b2gh = [_load_bias_col(nc, consts, d["b2gh"][r], f"b2g_{r}") for r in range(2)]
    pwinb = _load_bias_col(nc, consts, d["pwinb"], "pwinb")
    bqe = _load_bias_col(nc, consts, d["bqe"], "bqe")
    bke = _load_bias_col(nc, consts, d["bke"], "bke")
    pwoutb = _load_bias_col(nc, consts, d["pwoutb"], "pwoutb")

    # =========================== backbone ===========================
    with tc.tile_pool(name="convw", bufs=1) as convw, \
         tc.tile_pool(name="c1ps", bufs=4, space="PSUM") as c1ps, \
         tc.tile_pool(name="c2ps", bufs=4, space="PSUM") as c2ps:
        for r in range(2):
            # conv weights for this repeat (pre-transposed and packed on host,
            # one big DMA per K-tile: 6-12KB contiguous per partition line)
            c1sb = []
            c2sb = []
            for it in range(2):
                w1 = convw.tile([128, 6 * 256], FR, tag="c1w", bufs=4, name="w1")
                nc.sync.dma_start(out=w1[:], in_=d["c1w"][r, it])
                c1sb.append(w1)
                w2 = convw.tile([128, 6 * 512], BF, tag="c2w", bufs=4, name="w2")
                nc.sync.dma_start(out=w2[:], in_=d["c2w"][r, it])
                c2sb.append(w2)

            # h1' = ELU(x)+1 over the full padded tile (pads stay exactly 1)
            h1 = []
            for t in range(2):
                h = bb.tile([128, PTOT], FR, name=f"h1_{t}", tag=f"h1_{t}", bufs=1)
                for (r0_, r1_) in ((0, 578), (578, PTOT)):
                    tr = bb.tile([128, 578], F32, name="tr", tag="btr", bufs=3)
                    te = bb.tile([128, 578], F32, name="te", tag="bte", bufs=3)
                    w = r1_ - r0_
                    nc.vector.tensor_scalar(tr[:, 0:w], x_cur[t][:, r0_:r1_], 0.0, None, op0=ALU.max)
                    nc.scalar.activation(te[:, 0:w], x_cur[t][:, r0_:r1_], ACTF.Exp)
                    nc.vector.scalar_tensor_tensor(
                        h[:, r0_:r1_], te[:, 0:w], 1.0, tr[:, 0:w], op0=ALU.min, op1=ALU.add)
                h1.append(h)

            # conv1 -> h2' = ELU(y1 + b1_eff)+1
            h2 = []
            for t in range(2):
                h = bb.tile([128, PTOT], BF, name=f"h2_{t}", tag=f"h2_{t}", bufs=1)
                h2.append(h)
            for (s0, e0) in CHUNKS:
                for ot in range(2):
                    ps = c1ps.tile([128, CHUNK], F32, tag="c1")
                    n = 0
                    for tap in range(6):
                        for it in range(2):
                            nc.tensor.matmul(
                                ps[:],
                                (c1sb[it][:, tap * 256 + ot * 128:tap * 256 + (ot + 1) * 128]),
                                (h1[it][:, s0 + SHIFTS[tap]:e0 + SHIFTS[tap]]),
                                start=(n == 0), stop=(n == 11))
                            n += 1
                    tr = bb.tile([128, CHUNK], F32, name="ctr", tag="bctr", bufs=3)
                    te = bb.tile([128, CHUNK], F32, name="cte", tag="bcte", bufs=3)
                    nc.vector.tensor_scalar(tr[:], ps[:], b1[r][ot], 0.0, op0=ALU.add, op1=ALU.max)
                    nc.scalar.activation(te[:], ps[:], ACTF.Exp, bias=b1[r][ot])
                    nc.vector.scalar_tensor_tensor(
                        h2[ot][:, s0:e0], te[:], 1.0, tr[:], op0=ALU.min, op1=ALU.add)

            # pads of h2' must be exactly 1 -- written AFTER the chunk writes
            # (the chunks cover the in-row pad columns with garbage)
            for t in range(2):
                nc.vector.tensor_copy(h2[t][:, 0:P0], ones64[:, 0:P0])
                nc.vector.tensor_copy(
                    pad3(h2[t])[:, 2:33, 0:2],
                    ones64[:, 0:62].rearrange("p (r c) -> p r c", r=31, c=2))

            # conv2 -> GLU -> x_new. On the last repeat the result (and its
            # elu+1) live in the persistent pool: attention reads them via
            # strided 3D views, so no compacting copies are needed.
            x_new = []
            xe_pad = []
            for t in range(2):
                if r == 0:
                    xn = bb.tile([128, PTOT], F32, name=f"xn{t}", tag=f"xp{t}", bufs=2)
                else:
                    xn = wq_pool.tile([128, PTOT], F32, name=f"xfin{t}", tag=f"xfin{t}")
                    xep = wq_pool.tile([128, PTOT], FR, name=f"xep{t}", tag=f"xep{t}")
                    xe_pad.append(xep)
                x_new.append(xn)
            for (s0, e0) in CHUNKS:
                pss = []
                for ot in range(4):
                    ps = c2ps.tile([128, CHUNK], F32, tag="c2")
                    n = 0
                    for tap in range(6):
                        for it in range(2):
                            nc.tensor.matmul(
                                ps[:],
                                (c2sb[it][:, tap * 512 + ot * 128:tap * 512 + (ot + 1) * 128]),
                                (h2[it][:, s0 + SHIFTS[tap]:e0 + SHIFTS[tap]]),
                                start=(n == 0), stop=(n == 11))
                            n += 1
                    pss.append(ps)
                for t in range(2):
                    # sigmoid(g) = 0.5*(1 + tanh(g/2)); x += a * sigmoid(g)
                    th = bb.tile([128, CHUNK], F32, name="th", tag="th", bufs=3)
                    nc.scalar.activation(th[:], pss[2 + t][:], ACTF.Tanh,
                                         bias=b2gh[r][t], scale=0.5)
                    ah = bb.tile([128, CHUNK], F32, name="ah", tag="ah", bufs=3)
                    nc.vector.tensor_scalar(ah[:], pss[t][:], b2a[r][t], 0.5,
                                            op0=ALU.add, op1=ALU.mult)
                    gl = bb.tile([128, CHUNK], F32, name="gl", tag="gl", bufs=3)
                    nc.vector.scalar_tensor_tensor(gl[:], th[:], 1.0, ah[:],
                                                   op0=ALU.add, op1=ALU.mult)
                    nc.vector.tensor_tensor(x_new[t][:, s0:e0], x_cur[t][:, s0:e0],
                                            gl[:], op=ALU.add)
                    if r == 1:
                        # fused: xe' = ELU(x_new)+1 per chunk (overlaps conv2)
                        tr = bb.tile([128, CHUNK], F32, name="xtr", tag="bctr", bufs=3)
                        te = bb.tile([128, CHUNK], F32, name="xte", tag="bcte", bufs=3)
                        nc.vector.tensor_scalar(tr[:], x_new[t][:, s0:e0], 0.0, None, op0=ALU.max)
                        nc.scalar.activation(te[:], x_new[t][:, s0:e0], ACTF.Exp)
                        nc.vector.scalar_tensor_tensor(
                            xe_pad[t][:, s0:e0], te[:], 1.0, tr[:], op0=ALU.min, op1=ALU.add)
            if r == 0:
                for t in range(2):
                    nc.vector.memset(x_new[t][:, 0:P0], 0.0)
                    nc.vector.memset(pad3(x_new[t])[:, 2:33, 0:2], 0.0)
            x_cur = x_new

    # =========================== attention ===========================
    bb_stack.close()
    # interior 3D views of the persistent padded tiles
    res3 = [pad3(x_cur[t])[:, 1:33, 2:34] for t in range(2)]       # residual
    xe3 = [pad3(xe_pad[t])[:, 1:33, 2:34] for t in range(2)]       # elu(res)+1

    def xe_cols(t, c0, c1):
        assert c0 % 32 == 0 and c1 % 32 == 0
        return xe3[t][:, c0 // 32:c1 // 32, :]

    pwin_sb, wq_sb, wk_sb, wv_sb, pwout_sb = load_apw()
    causal, ind4 = build_masks()
    heads = ctx.enter_context(tc.tile_pool(name="heads", bufs=1))
    proj_stack = ExitStack()
    proj = proj_stack.enter_context(tc.tile_pool(name="proj", bufs=1))

    HCH = [(0, 512), (512, 1024)]
    with tc.tile_pool(name="prep_ps", bufs=4, space="PSUM") as prep_ps, \
         tc.tile_pool(name="vt_ps", bufs=2, space="PSUM") as vt_ps:
        # pw_in -> x' (elu+1 of pointwise output)
        xp = [proj.tile([128, S], FR, name=f"xpa{t}", tag=f"xpa{t}", bufs=1) for t in range(2)]
        for ot in range(2):
            for (c0, c1) in HCH:
                ps = prep_ps.tile([128, 512], F32, tag="pp")
                for it in range(2):
                    nc.tensor.matmul(
                        ps[:], (pwin_sb[it][:, ot * 128:(ot + 1) * 128]),
                        xe_cols(it, c0, c1), start=(it == 0), stop=(it == 1))
                tr = proj.tile([128, 512], F32, name="ctr", tag="ctr", bufs=3)
                te = proj.tile([128, 512], F32, name="cte", tag="cte", bufs=3)
                nc.vector.tensor_scalar(tr[:], ps[:], pwinb[ot], 0.0, op0=ALU.add, op1=ALU.max)
                nc.scalar.activation(te[:], ps[:], ACTF.Exp, bias=pwinb[ot])
                nc.vector.scalar_tensor_tensor(
                    xp[ot][:, c0:c1], te[:], 1.0, tr[:], op0=ALU.min, op1=ALU.add)

        # q, k projections (biased, scale folded into score exp later)
        q_sb = [heads.tile([128, S], FR, name=f"q{t}", tag=f"q{t}", bufs=1) for t in range(2)]
        k_sb = [heads.tile([128, S], FR, name=f"k{t}", tag=f"k{t}", bufs=1) for t in range(2)]
        for (wsb, osb, bias) in ((wq_sb, q_sb, bqe), (wk_sb, k_sb, bke)):
            for ot in range(2):
                for (c0, c1) in HCH:
                    ps = prep_ps.tile([128, 512], F32, tag="pp")
                    for it in range(2):
                        nc.tensor.matmul(
                            ps[:], (wsb[it][:, ot * 128:(ot + 1) * 128]),
                            (xp[it][:, c0:c1]), start=(it == 0), stop=(it == 1))
                    nc.vector.tensor_scalar(osb[ot][:, c0:c1], ps[:], bias[ot], None, op0=ALU.add)

        # matmul operands need partition base in {0,32,64}; heads 3 and 7 sit
        # at base 96, so relocate those two into one extra tile pair via DMA.
        q37 = heads.tile([64, S], FR, name="q37", tag="q37", bufs=1)
        k37 = heads.tile([64, S], FR, name="k37", tag="k37", bufs=1)
        for ti in range(2):
            nc.sync.dma_start(out=q37[ti * 32:(ti + 1) * 32, :], in_=q_sb[ti][96:128, :])
            nc.sync.dma_start(out=k37[ti * 32:(ti + 1) * 32, :], in_=k_sb[ti][96:128, :])

        def q_head(h):
            ti, b = divmod(h, 4)
            if b < 3:
                return q_sb[ti][b * 32:(b + 1) * 32, :]
            return q37[ti * 32:(ti + 1) * 32, :]

        def k_head(h):
            ti, b = divmod(h, 4)
            if b < 3:
                return k_sb[ti][b * 32:(b + 1) * 32, :]
            return k37[ti * 32:(ti + 1) * 32, :]

        # S2 broadcast tile: (wv.sum(1) - bv) replicated to 128 partitions
        s2row = proj.tile([1, 256], FR, name="s2row", tag="s2row", bufs=1)
        nc.sync.dma_start(out=s2row[:], in_=d["s2row"][:, :])
        ps_s2 = vt_ps.tile([128, 256], F32, tag="s2")
        nc.tensor.matmul(ps_s2[:], (ones_row[:]), (s2row[:]), start=True, stop=True)
        s2_sb = proj.tile([128, 256], F32, name="s2sb", tag="s2sb", bufs=1)
        nc.scalar.activation(s2_sb[:], ps_s2[:], ACTF.Copy)

        # V^T projection (x' as stationary), head-interleaved with ones cols
        vt = []
        for st in range(8):
            ps = vt_ps.tile([128, 256], F32, tag="vt")
            for it in range(2):
                nc.tensor.matmul(
                    ps[:], (xp[it][:, st * 128:(st + 1) * 128]),
                    (wv_sb[it][:]), start=(it == 0), stop=(it == 1))
            v = heads.tile([128, NH * 33], FR, name="vt", tag="vt", bufs=8)
            v3 = v.rearrange("p (h e) -> p h e", h=NH, e=33)
            nc.vector.tensor_tensor(
                v3[:, :, 0:32],
                ps[:].rearrange("p (h e) -> p h e", h=NH, e=32),
                s2_sb[:].rearrange("p (h e) -> p h e", h=NH, e=32),
                op=ALU.subtract)
            nc.vector.tensor_copy(v3[:, :, 32:33],
                                  ones8[:].rearrange("p (h e) -> p h e", e=1))
            vt.append(v)

    # heads + per-group normalization tail, sharing one PSUM budget:
    # tags: "sc" (scores / pw_out) and "o" (PV out / recip broadcast), 2 bufs
    # each of 2 banks -> 8 banks total. The group-t tail is emitted right
    # after its 4 heads so it overlaps the other group's compute.
    attn_out = [wq_pool.tile([128, S], F32, name=f"ao{t}", tag=f"ao{t}", bufs=1) for t in range(2)]
    sums4 = [wq_pool.tile([4, S], F32, name=f"sums{t}", tag=f"sums{t}", bufs=1) for t in range(2)]
    proj_stack.close()
    acts = ctx.enter_context(tc.tile_pool(name="tail", bufs=1))
    ho = []
    with tc.tile_pool(name="sc_ps", bufs=2, space="PSUM") as sc_ps, \
         tc.tile_pool(name="o_ps", bufs=2, space="PSUM") as o_ps, \
         tc.tile_pool(name="expp", bufs=8) as expp:
        for t in range(2):
            for h in range(4 * t, 4 * t + 4):
                ti, base = divmod(h, 4)
                eT = []
                for j in range(8):
                    L = S - j * 128
                    ps = sc_ps.tile([128, 1024], F32, tag="sc", name="ps")
                    p0 = 0
                    while p0 < L:
                        pl = min(512, L - p0)
                        if L - p0 - pl == 128:
                            pl = 384  # keep the tail piece >= 256 where possible
                        nc.tensor.matmul(
                            ps[:, p0:p0 + pl],
                            (k_head(h)[:, j * 128:(j + 1) * 128]),
                            (q_head(h)[:, j * 128 + p0:j * 128 + p0 + pl]),
                            start=True, stop=True)
                        p0 += pl
                    e = expp.tile([128, 1024], FR, tag="expT", name="e")
                    nc.scalar.activation(e[:, 0:L], ps[:, 0:L], ACTF.Exp, scale=INV_SQRT_DK)
                    nc.vector.tensor_tensor(e[:, 0:128], e[:, 0:128], causal[:], op=ALU.mult)
                    eT.append(e)
                # PV with fused denominator row
                ops = o_ps.tile([33, S], F32, tag="o", name="ops")
                for c2 in range(2):
                    cs, ce = c2 * 512, (c2 + 1) * 512
                    jmax = min(4 * c2 + 3, 7)
                    for j in range(jmax + 1):
                        qs = max(cs, j * 128)
                        nc.tensor.matmul(
                            ops[:, qs:ce],
                            (vt[j][:, h * 33:(h + 1) * 33]),
                            (eT[j][:, qs - j * 128:ce - j * 128]),
                            start=(j == 0), stop=(j == jmax))
                ohb = heads.tile([33, S], F32, name="ohb", tag="ohb", bufs=3)
                nc.scalar.activation(ohb[:], ops[:], ACTF.Copy)
                nc.sync.dma_start(out=attn_out[ti][base * 32:(base + 1) * 32, :], in_=ohb[0:32, :])
                nc.sync.dma_start(out=sums4[ti][base:base + 1, :], in_=ohb[32:33, :])

            # ---- group-t normalization + ELU(+1) ----
            recip = acts.tile([4, S], F32, name="recip", tag="recip", bufs=2)
            scr = acts.tile([4, S], F32, name="rscr", tag="rscr", bufs=2)
            nc.vector.reciprocal_approx_accurate(recip[:], sums4[t][:], scr[:])
            recip_r = acts.tile([4, S], FR, name="recip_r", tag="recip_r", bufs=2)
            nc.vector.tensor_copy(recip_r[:], recip[:])
            psr = o_ps.tile([128, S], F32, tag="o", name="psr")
            for (c0, c1) in HCH:
                nc.tensor.matmul(psr[:, c0:c1], (ind4[:]),
                                 (recip_r[:, c0:c1]), start=True, stop=True)
            onrm = acts.tile([128, S], F32, name="onrm", tag="onrm", bufs=2)
            nc.vector.tensor_tensor(onrm[:], attn_out[t][:], psr[:], op=ALU.mult)
            tr = acts.tile([128, S], F32, name="tr", tag="tr", bufs=2)
            te = acts.tile([128, S], F32, name="te", tag="te", bufs=2)
            hh = acts.tile([128, S], FR, name=f"ho{t}", tag=f"ho{t}", bufs=1)
            nc.vector.tensor_scalar(tr[:], onrm[:], 0.0, None, op0=ALU.max)
            nc.scalar.activation(te[:], onrm[:], ACTF.Exp)
            nc.vector.scalar_tensor_tensor(hh[:], te[:], 1.0, tr[:], op0=ALU.min, op1=ALU.add)
            ho.append(hh)

        # ---- output pointwise block + residual ----
        for ot in range(2):
            for (c0, c1) in HCH:
                ps = o_ps.tile([128, 512], F32, tag="o", name="pops")
                for it in range(2):
                    nc.tensor.matmul(
                        ps[:], (pwout_sb[it][:, ot * 128:(ot + 1) * 128]),
                        (ho[it][:, c0:c1]), start=(it == 0), stop=(it == 1))
                tr = acts.tile([128, 512], F32, name="ctr", tag="ctr", bufs=3)
                te = acts.tile([128, 512], F32, name="cte", tag="cte", bufs=3)
                u = acts.tile([128, 512], F32, name="fu", tag="fu", bufs=2)
                fin = acts.tile([128, 512], F32, name="fin", tag="fin", bufs=2)
                nc.vector.tensor_scalar(tr[:], ps[:], pwoutb[ot], 0.0, op0=ALU.add, op1=ALU.max)
                nc.scalar.activation(te[:], ps[:], ACTF.Exp, bias=pwoutb[ot])
                nc.vector.scalar_tensor_tensor(u[:], te[:], 1.0, tr[:], op0=ALU.min, op1=ALU.add)
                nc.vector.scalar_tensor_tensor(
                    fin[:].rearrange("p (r c) -> p r c", r=16, c=32),
                    u[:].rearrange("p (r c) -> p r c", r=16, c=32), -1.0,
                    res3[ot][:, c0 // 32:c1 // 32, :],
                    op0=ALU.add, op1=ALU.add)
                nc.sync.dma_start(out=out_d[ot * 128:(ot + 1) * 128, c0:c1], in_=fin[:])


_CACHED_NC = None


def _get_nc():
    global _CACHED_NC
    if _CACHED_NC is None:
        _CACHED_NC = _build_program()
    return _CACHED_NC


def _prep_host(inputs):
    """Host-side packing: shard input over batch, pre-transpose weights,
    fold the elu+1 corrections into effective biases."""
    f = np.float32
    rb_w_in = np.asarray(inputs["rb_w_in"], f)
    rb_w_out = np.asarray(inputs["rb_w_out"], f)
    wv = np.asarray(inputs["wv"], f)

    # [r, tap, i, o] -> packed [r, it, p(128), tap*o]
    c1t = rb_w_in.transpose(0, 3, 4, 2, 1).reshape(2, 6, C, C)
    c1w = np.ascontiguousarray(
        c1t.reshape(2, 6, 2, 128, C).transpose(0, 2, 3, 1, 4).reshape(2, 2, 128, 6 * C))
    c2t = rb_w_out.transpose(0, 3, 4, 2, 1).reshape(2, 6, C, 2 * C)
    c2w = np.ascontiguousarray(
        c2t.reshape(2, 6, 2, 128, 2 * C).transpose(0, 2, 3, 1, 4).reshape(2, 2, 128, 12 * C)
        .astype(ml_dtypes.bfloat16))
    b1e = inputs["rb_b_in"] - rb_w_in.sum((2, 3, 4))
    b2e = inputs["rb_b_out"] - rb_w_out.sum((2, 3, 4))
    common = {
        "c1w": c1w,
        "c2w": c2w,
        "b1e": np.ascontiguousarray(b1e, f),
        "b2a": np.ascontiguousarray(b2e[:, :C], f),
        "b2gh": np.ascontiguousarray(0.5 * b2e[:, C:], f),
        "apw": np.ascontiguousarray(
            np.stack([np.asarray(inputs["pw_in_w"], f).T,
                      np.asarray(inputs["wq"], f).T,
                      np.asarray(inputs["wk"], f).T,
                      wv.T,
                      np.asarray(inputs["pw_out_w"], f).T])  # [5, c, o]
            .reshape(5, 2, 128, C).transpose(1, 2, 0, 3).reshape(2, 128, 5 * C)),
        "pwinb": np.asarray(inputs["pw_in_b"] - np.asarray(inputs["pw_in_w"], f).sum(1), f),
        "bqe": np.asarray(inputs["bq"] - np.asarray(inputs["wq"], f).sum(1), f),
        "bke": np.asarray(inputs["bk"] - np.asarray(inputs["wk"], f).sum(1), f),
        "s2row": np.ascontiguousarray((wv.sum(1) - np.asarray(inputs["bv"], f))[None, :]),
        "pwoutb": np.asarray(
            inputs["pw_out_b"] - np.asarray(inputs["pw_out_w"], f).sum(1), f),
    }
    common = {k: (v if v.dtype == ml_dtypes.bfloat16 else np.ascontiguousarray(v, f)) for k, v in common.items()}
    inp = np.asarray(inputs["input"], f)
    in_maps = []
    for c in range(8):
        m = dict(common)
        m["inp"] = np.ascontiguousarray(inp[c].reshape(C, S))
        in_maps.append(m)
    return in_maps


def kernel(**inputs) -> np.ndarray:
    nc = _get_nc()
    in_maps = _prep_host(inputs)
    res = run_bass_kernel_spmd(nc, in_maps, list(range(8)))
    out = np.stack([res.results[c]["out"].reshape(C, 32, 32) for c in range(8)])
    return out.astype(np.float32)


def run_traced(inputs):
    """For test.py: run with NTFF profiling, returns (output, exec_time_ns)."""
    import types
    import trn_agent_boot.trn_boot as tb
    hook = tb._ntff_profile_via_ctypes("/opt/axon/libaxon_pjrt.so")
    mod = types.ModuleType("antenv.axon_hooks")
    mod.get_axon_ntff_profile_hook = lambda: hook
    import antenv
    sys.modules["antenv.axon_hooks"] = mod
    antenv.axon_hooks = mod

    nc = _get_nc()
    in_maps = _prep_host(inputs)
    res = run_bass_kernel_spmd(nc, in_maps, list(range(8)), trace=True)
    out = np.stack([res.results[c]["out"].reshape(C, 32, 32) for c in range(8)])
    return out.astype(np.float32), res.exec_time_ns


# revision 23
# speedup vs baseline: 1.0079x; 1.0079x over previous
"""Trainium2 Bass kernel for nn_PixelBlock (causal-conv GLU backbone + causal
self-attention block). Data-parallel over batch: 8 images -> 8 NeuronCores.

Layout notes (per core, B=1):
- Activations are channel-major [C, S] with C on partitions (256 -> 2 tiles).
- For the 2x3 causal convs, spatial is kept in a padded layout: 33 rows
  (1 top-pad) x 34 cols (2 left-pad) = 1122 flat positions. Each conv tap is
  then a pure flat shift, so conv = 6 accumulated matmuls per K-tile.
- ELU(x) is computed as max(x,0) + min(exp(x),1) - 1. We carry h' = elu+1
  (so padding cells hold exactly 1.0) and fold the -1 into the next layer's
  bias via  b_eff = b - W @ ones  (computed on host).
- Sigmoid is computed via tanh (same ACT table set as exp -> no table swaps).
- Attention: scores are built transposed (sT[k,q] = k_h^T q_h), exp'd with
  the 1/sqrt(dk) scale folded into the ACT scale, diagonal 128-blocks masked
  with a 0/1 lower-triangle mask. PV uses V^T (computed directly by a
  transposed projection) augmented with a ones-column so the softmax
  denominators fall out of the same matmul. Output is normalized by a
  reciprocal broadcast through a tiny K=8 indicator matmul.
"""

import sys
import numpy as np
import ml_dtypes

sys.path.insert(0, "/opt/trn_rl_repo")

from contextlib import ExitStack

import concourse.bass as bass
import concourse.tile as tile
from concourse import bacc, mybir
from concourse.bass_utils import run_bass_kernel_spmd

F32 = mybir.dt.float32
FR = mybir.dt.float32r
BF = mybir.dt.bfloat16
ALU = mybir.AluOpType
ACTF = mybir.ActivationFunctionType

C = 256
S = 1024
NH = 8
DK = 32
PW = 34          # padded row width (2 left pad + 32)
NROW = 33        # 1 top pad + 32 rows
PTOT = NROW * PW  # 1122
P0 = 36          # first valid output position (row 1, col 2)
CHUNK = 362      # conv output chunk (3 * 362 = 1086 = PTOT - P0)
CHUNKS = [(P0 + i * CHUNK, P0 + (i + 1) * CHUNK) for i in range(3)]
# tap shifts for (di, dj) in row-major order, shift = (di-1)*34 + (dj-2)
SHIFTS = [-36, -35, -34, -2, -1, 0]
INV_SQRT_DK = 1.0 / np.sqrt(32.0)


def _build_program():
    nc = bacc.Bacc("TRN2", target_bir_lowering=False, debug=False, num_devices=8)

    d = {}
    def din(name, shape, dt=F32):
        d[name] = nc.dram_tensor(name, list(shape), dt, kind="ExternalInput").ap()
    din("inp", (C, S))
    din("c1w", (2, 2, 128, 6 * C), BF)   # [r, it, p, tap*o] packed
    din("c2w", (2, 2, 128, 12 * C), BF)  # [r, it, p, tap*o] packed
    din("b1e", (2, C))              # rb_b_in - sum(w)
    din("b2a", (2, C))              # a-part bias (already * 1.0)
    din("b2gh", (2, C))             # 0.5 * g-part bias
    din("apw", (2, 128, 5 * C), FR)       # [it, p, {pwin,wq,wk,wv,pwout}*o]
    din("pwinb", (C,))
    din("bqe", (C,))
    din("bke", (C,))
    din("s2row", (1, C), FR)        # wv.sum(1) - bv
    din("pwoutb", (C,))
    out_d = nc.dram_tensor("out", [C, S], F32, kind="ExternalOutput").ap()

    with tile.TileContext(nc) as tc, ExitStack() as ctx:
        _kernel_body(ctx, tc, nc, d, out_d)

    nc.compile()
    return nc


def _load_bias_col(nc, pool, dram_ap_1d, tag):
    """[256] dram vector -> two [128,1] sbuf column tiles."""
    tiles = []
    for t in range(2):
        b = pool.tile([128, 1], F32, tag=f"{tag}{t}")
        nc.sync.dma_start(out=b[:], in_=dram_ap_1d[t * 128:(t + 1) * 128].unsqueeze(1))
        tiles.append(b)
    return tiles


def _kernel_body(ctx, tc, nc, d, out_d):
    consts = ctx.enter_context(tc.tile_pool(name="consts", bufs=1))
    wq_pool = ctx.enter_context(tc.tile_pool(name="wq", bufs=1))

    # ---- constants (gpsimd-built masks are emitted in the attention
    # section: gpsimd takes ~26us to boot and must stay off the critical
    # path of the backbone's DVE stream) ----
    ones_f = consts.tile([1, 128], F32)
    nc.vector.memset(ones_f[:], 1.0)
    ones_row = consts.tile([1, 128], FR)
    nc.vector.tensor_copy(ones_row[:], ones_f[:])
    ones8 = consts.tile([128, 8], F32)
    nc.vector.memset(ones8[:], 1.0)
    ones64 = consts.tile([128, 64], F32)
    nc.vector.memset(ones64[:], 1.0)

    def build_masks():
        causal = consts.tile([128, 128], F32, name="causal")   # 1 where k <= q
        nc.gpsimd.memset(causal[:], 1.0)
        nc.gpsimd.affine_select(
            out=causal[:], in_=causal[:], compare_op=ALU.is_ge, fill=0.0,
            base=0, channel_multiplier=-1, pattern=[[1, 128]])
        ind4_f = consts.tile([4, 128], F32, name="ind4_f")     # 1 where m//32 == k
        nc.gpsimd.memset(ind4_f[:], 1.0)
        nc.gpsimd.affine_select(
            out=ind4_f[:], in_=ind4_f[:], compare_op=ALU.is_ge, fill=0.0,
            base=0, channel_multiplier=-32, pattern=[[1, 128]])
        nc.gpsimd.affine_select(
            out=ind4_f[:], in_=ind4_f[:], compare_op=ALU.is_ge, fill=0.0,
            base=31, channel_multiplier=32, pattern=[[-1, 128]])
        ind4 = consts.tile([4, 128], FR, name="ind4")
        nc.vector.tensor_copy(ind4[:], ind4_f[:])
        return causal, ind4


    # ---- attention / pointwise weights (packed, persistent) ----
    def load_apw():
        groups = [[], [], [], [], []]
        for t in range(2):
            w = wq_pool.tile([128, 5 * C], FR, tag=f"apw{t}", name=f"apw{t}")
            nc.sync.dma_start(out=w[:], in_=d["apw"][t])
            for g in range(5):
                groups[g].append(w[:, g * C:(g + 1) * C])
        return groups

    # ---- input -> padded layout ----
    def pad3(t2d):
        return t2d.rearrange("p (r c) -> p r c", r=NROW, c=PW)

    bb_stack = ExitStack()
    bb = bb_stack.enter_context(tc.tile_pool(name="bb", bufs=1))
    x_cur = []
    for t in range(2):
        x0 = bb.tile([128, PTOT], F32, name=f"x0_{t}", tag=f"xp{t}", bufs=2)
        nc.vector.memset(x0[:, 0:P0], 0.0)
        nc.vector.memset(pad3(x0)[:, 2:33, 0:2], 0.0)
        for hf in range(4):
            stg = bb.tile([128, 256], F32, name=f"stg{t}{hf}", tag="stg", bufs=8)
            nc.sync.dma_start(out=stg[:], in_=d["inp"][t * 128:(t + 1) * 128, hf * 256:(hf + 1) * 256])
            nc.vector.tensor_copy(
                pad3(x0)[:, 1 + 8 * hf:9 + 8 * hf, 2:34],
                stg[:].rearrange("p (r c) -> p r c", r=8, c=32))
        x_cur.append(x0)

    # ---- biases (emitted after input/weight DMAs; needed post-conv1) ----
    b1 = [_load_bias_col(nc, consts, d["b1e"][r], f"b1_{r}") for r in range(2)]
    b2a = [_load_bias_col(nc, consts, d["b2a"][r], f"b2a_{r}") for r in range(2)]
    b2gh = [_load_bias_col(nc, consts, d["b2gh"][r], f"b2g_{r}") for r in range(2)]
    pwinb = _load_bias_col(nc, consts, d["pwinb"], "pwinb")
    bqe = _load_bias_col(nc, consts, d["bqe"], "bqe")
    bke = _load_bias_col(nc, consts, d["bke"], "bke")
    pwoutb = _load_bias_col(nc, consts, d["pwoutb"], "pwoutb")

    # =========================== backbone ===========================
    with tc.tile_pool(name="convw", bufs=1) as convw, \
         tc.tile_pool(name="c1ps", bufs=4, space="PSUM") as c1ps, \
         tc.tile_pool(name="c2ps", bufs=4, space="PSUM") as c2ps:
        for r in range(2):
            # conv weights for this repeat (pre-transposed and packed on host,
            # one big DMA per K-tile: 6-12KB contiguous per partition line)
            c1sb = []
            c2sb = []
            for it in range(2):
                w1 = convw.tile([128, 6 * 256], BF, tag="c1w", bufs=4, name="w1")
                nc.sync.dma_start(out=w1[:, 0:768], in_=d["c1w"][r, it, :, 0:768])
                nc.sync.dma_start(out=w1[:, 768:1536], in_=d["c1w"][r, it, :, 768:1536])
                c1sb.append(w1)
                w2 = convw.tile([128, 6 * 512], BF, tag="c2w", bufs=4, name="w2")
                nc.sync.dma_start(out=w2[:, 0:1536], in_=d["c2w"][r, it, :, 0:1536])
                nc.sync.dma_start(out=w2[:, 1536:3072], in_=d["c2w"][r, it, :, 1536:3072])
                c2sb.append(w2)

            # h1' = ELU(x)+1 over the full padded tile (pads stay exactly 1)
            h1 = []
            for t in range(2):
                h = bb.tile([128, PTOT], BF, name=f"h1_{t}", tag=f"h1_{t}", bufs=1)
                for (r0_, r1_) in ((0, 578), (578, PTOT)):
                    tr = bb.tile([128, 578], F32, name="tr", tag="btr", bufs=3)
                    te = bb.tile([128, 578], F32, name="te", tag="bte", bufs=3)
                    w = r1_ - r0_
                    nc.vector.tensor_scalar(tr[:, 0:w], x_cur[t][:, r0_:r1_], 0.0, None, op0=ALU.max)
                    nc.scalar.activation(te[:, 0:w], x_cur[t][:, r0_:r1_], ACTF.Exp)
                    nc.vector.scalar_tensor_tensor(
                        h[:, r0_:r1_], te[:, 0:w], 1.0, tr[:, 0:w], op0=ALU.min, op1=ALU.add)
                h1.append(h)

            # conv1 -> h2' = ELU(y1 + b1_eff)+1
            h2 = []
            for t in range(2):
                h = bb.tile([128, PTOT], BF, name=f"h2_{t}", tag=f"h2_{t}", bufs=1)
                h2.append(h)
            for (s0, e0) in CHUNKS:
                for ot in range(2):
                    ps = c1ps.tile([128, CHUNK], F32, tag="c1")
                    n = 0
                    for tap in range(6):
                        for it in range(2):
                            nc.tensor.matmul(
                                ps[:],
                                (c1sb[it][:, tap * 256 + ot * 128:tap * 256 + (ot + 1) * 128]),
                                (h1[it][:, s0 + SHIFTS[tap]:e0 + SHIFTS[tap]]),
                                start=(n == 0), stop=(n == 11))
                            n += 1
                    tr = bb.tile([128, CHUNK], F32, name="ctr", tag="bctr", bufs=3)
                    te = bb.tile([128, CHUNK], F32, name="cte", tag="bcte", bufs=3)
                    nc.vector.tensor_scalar(tr[:], ps[:], b1[r][ot], 0.0, op0=ALU.add, op1=ALU.max)
                    nc.scalar.activation(te[:], ps[:], ACTF.Exp, bias=b1[r][ot])
                    nc.vector.scalar_tensor_tensor(
                        h2[ot][:, s0:e0], te[:], 1.0, tr[:], op0=ALU.min, op1=ALU.add)

            # pads of h2' must be exactly 1 -- written AFTER the chunk writes
            # (the chunks cover the in-row pad columns with garbage)
            for t in range(2):
                nc.vector.tensor_copy(h2[t][:, 0:P0], ones64[:, 0:P0])
                nc.vector.tensor_copy(
                    pad3(h2[t])[:, 2:33, 0:2],
                    ones64[:, 0:62].rearrange("p (r c) -> p r c", r=31, c=2))

            # conv2 -> GLU -> x_new. On the last repeat the result (and its
            # elu+1) live in the persistent pool: attention reads them via
            # strided 3D views, so no compacting copies are needed.
            x_new = []
            xe_pad = []
            for t in range(2):
                if r == 0:
                    xn = bb.tile([128, PTOT], F32, name=f"xn{t}", tag=f"xp{t}", bufs=2)
                else:
                    xn = wq_pool.tile([128, PTOT], F32, name=f"xfin{t}", tag=f"xfin{t}")
                    xep = wq_pool.tile([128, PTOT], FR, name=f"xep{t}", tag=f"xep{t}")
                    xe_pad.append(xep)
                x_new.append(xn)
            for (s0, e0) in CHUNKS:
                pss = []
                for ot in range(4):
                    ps = c2ps.tile([128, CHUNK], F32, tag="c2")
                    n = 0
                    for tap in range(6):
                        for it in range(2):
                            nc.tensor.matmul(
                                ps[:],
                                (c2sb[it][:, tap * 512 + ot * 128:tap * 512 + (ot + 1) * 128]),
                                (h2[it][:, s0 + SHIFTS[tap]:e0 + SHIFTS[tap]]),
                                start=(n == 0), stop=(n == 11))
                            n += 1
                    pss.append(ps)
                for t in range(2):
                    # sigmoid(g) = 0.5*(1 + tanh(g/2)); x += a * sigmoid(g)
                    th = bb.tile([128, CHUNK], F32, name="th", tag="th", bufs=3)
                    nc.scalar.activation(th[:], pss[2 + t][:], ACTF.Tanh,
                                         bias=b2gh[r][t], scale=0.5)
                    ah = bb.tile([128, CHUNK], F32, name="ah", tag="ah", bufs=3)
                    nc.vector.tensor_scalar(ah[:], pss[t][:], b2a[r][t], 0.5,
                                            op0=ALU.add, op1=ALU.mult)
                    gl = bb.tile([128, CHUNK], F32, name="gl", tag="gl", bufs=3)
                    nc.vector.scalar_tensor_tensor(gl[:], th[:], 1.0, ah[:],
                                                   op0=ALU.add, op1=ALU.mult)
                    nc.vector.tensor_tensor(x_new[t][:, s0:e0], x_cur[t][:, s0:e0],
                                            gl[:], op=ALU.add)
                    if r == 1:
                        # fused: xe' = ELU(x_new)+1 per chunk (overlaps conv2)
                        tr = bb.tile([128, CHUNK], F32, name="xtr", tag="bctr", bufs=3)
                        te = bb.tile([128, CHUNK], F32, name="xte", tag="bcte", bufs=3)
                        nc.vector.tensor_scalar(tr[:], x_new[t][:, s0:e0], 0.0, None, op0=ALU.max)
                        nc.scalar.activation(te[:], x_new[t][:, s0:e0], ACTF.Exp)
                        nc.vector.scalar_tensor_tensor(
                            xe_pad[t][:, s0:e0], te[:], 1.0, tr[:], op0=ALU.min, op1=ALU.add)
            if r == 0:
                for t in range(2):
                    nc.vector.memset(x_new[t][:, 0:P0], 0.0)
                    nc.vector.memset(pad3(x_new[t])[:, 2:33, 0:2], 0.0)
            x_cur = x_new

    # =========================== attention ===========================
    bb_stack.close()
    # interior 3D views of the persistent padded tiles
    res3 = [pad3(x_cur[t])[:, 1:33, 2:34] for t in range(2)]       # residual
    xe3 = [pad3(xe_pad[t])[:, 1:33, 2:34] for t in range(2)]       # elu(res)+1

    def xe_cols(t, c0, c1):
        assert c0 % 32 == 0 and c1 % 32 == 0
        return xe3[t][:, c0 // 32:c1 // 32, :]

    pwin_sb, wq_sb, wk_sb, wv_sb, pwout_sb = load_apw()
    causal, ind4 = build_masks()
    heads = ctx.enter_context(tc.tile_pool(name="heads", bufs=1))
    proj_stack = ExitStack()
    proj = proj_stack.enter_context(tc.tile_pool(name="proj", bufs=1))

    HCH = [(0, 512), (512, 1024)]
    with tc.tile_pool(name="prep_ps", bufs=4, space="PSUM") as prep_ps, \
         tc.tile_pool(name="vt_ps", bufs=2, space="PSUM") as vt_ps:
        # pw_in -> x' (elu+1 of pointwise output)
        xp = [proj.tile([128, S], FR, name=f"xpa{t}", tag=f"xpa{t}", bufs=1) for t in range(2)]
        for ot in range(2):
            for (c0, c1) in HCH:
                ps = prep_ps.tile([128, 512], F32, tag="pp")
                for it in range(2):
                    nc.tensor.matmul(
                        ps[:], (pwin_sb[it][:, ot * 128:(ot + 1) * 128]),
                        xe_cols(it, c0, c1), start=(it == 0), stop=(it == 1))
                tr = proj.tile([128, 512], F32, name="ctr", tag="ctr", bufs=3)
                te = proj.tile([128, 512], F32, name="cte", tag="cte", bufs=3)
                nc.vector.tensor_scalar(tr[:], ps[:], pwinb[ot], 0.0, op0=ALU.add, op1=ALU.max)
                nc.scalar.activation(te[:], ps[:], ACTF.Exp, bias=pwinb[ot])
                nc.vector.scalar_tensor_tensor(
                    xp[ot][:, c0:c1], te[:], 1.0, tr[:], op0=ALU.min, op1=ALU.add)

        # q, k projections (biased, scale folded into score exp later)
        q_sb = [heads.tile([128, S], FR, name=f"q{t}", tag=f"q{t}", bufs=1) for t in range(2)]
        k_sb = [heads.tile([128, S], FR, name=f"k{t}", tag=f"k{t}", bufs=1) for t in range(2)]
        for (wsb, osb, bias) in ((wq_sb, q_sb, bqe), (wk_sb, k_sb, bke)):
            for ot in range(2):
                for (c0, c1) in HCH:
                    ps = prep_ps.tile([128, 512], F32, tag="pp")
                    for it in range(2):
                        nc.tensor.matmul(
                            ps[:], (wsb[it][:, ot * 128:(ot + 1) * 128]),
                            (xp[it][:, c0:c1]), start=(it == 0), stop=(it == 1))
                    nc.vector.tensor_scalar(osb[ot][:, c0:c1], ps[:], bias[ot], None, op0=ALU.add)

        # matmul operands need partition base in {0,32,64}; heads 3 and 7 sit
        # at base 96, so relocate those two into one extra tile pair via DMA.
        q37 = heads.tile([64, S], FR, name="q37", tag="q37", bufs=1)
        k37 = heads.tile([64, S], FR, name="k37", tag="k37", bufs=1)
        for ti in range(2):
            nc.sync.dma_start(out=q37[ti * 32:(ti + 1) * 32, :], in_=q_sb[ti][96:128, :])
            nc.sync.dma_start(out=k37[ti * 32:(ti + 1) * 32, :], in_=k_sb[ti][96:128, :])

        def q_head(h):
            ti, b = divmod(h, 4)
            if b < 3:
                return q_sb[ti][b * 32:(b + 1) * 32, :]
            return q37[ti * 32:(ti + 1) * 32, :]

        def k_head(h):
            ti, b = divmod(h, 4)
            if b < 3:
                return k_sb[ti][b * 32:(b + 1) * 32, :]
            return k37[ti * 32:(ti + 1) * 32, :]

        # S2 broadcast tile: (wv.sum(1) - bv) replicated to 128 partitions
        s2row = proj.tile([1, 256], FR, name="s2row", tag="s2row", bufs=1)
        nc.sync.dma_start(out=s2row[:], in_=d["s2row"][:, :])
        ps_s2 = vt_ps.tile([128, 256], F32, tag="s2")
        nc.tensor.matmul(ps_s2[:], (ones_row[:]), (s2row[:]), start=True, stop=True)
        s2_sb = proj.tile([128, 256], F32, name="s2sb", tag="s2sb", bufs=1)
        nc.scalar.activation(s2_sb[:], ps_s2[:], ACTF.Copy)

        # V^T projection (x' as stationary), head-interleaved with ones cols
        vt = []
        for st in range(8):
            ps = vt_ps.tile([128, 256], F32, tag="vt")
            for it in range(2):
                nc.tensor.matmul(
                    ps[:], (xp[it][:, st * 128:(st + 1) * 128]),
                    (wv_sb[it][:]), start=(it == 0), stop=(it == 1))
            v = heads.tile([128, NH * 33], FR, name="vt", tag="vt", bufs=8)
            v3 = v.rearrange("p (h e) -> p h e", h=NH, e=33)
            nc.vector.tensor_tensor(
                v3[:, :, 0:32],
                ps[:].rearrange("p (h e) -> p h e", h=NH, e=32),
                s2_sb[:].rearrange("p (h e) -> p h e", h=NH, e=32),
                op=ALU.subtract)
            nc.vector.tensor_copy(v3[:, :, 32:33],
                                  ones8[:].rearrange("p (h e) -> p h e", e=1))
            vt.append(v)

    # heads + per-group normalization tail, sharing one PSUM budget:
    # tags: "sc" (scores / pw_out) and "o" (PV out / recip broadcast), 2 bufs
    # each of 2 banks -> 8 banks total. The group-t tail is emitted right
    # after its 4 heads so it overlaps the other group's compute.
    attn_out = [wq_pool.tile([128, S], F32, name=f"ao{t}", tag=f"ao{t}", bufs=1) for t in range(2)]
    sums4 = [wq_pool.tile([4, S], F32, name=f"sums{t}", tag=f"sums{t}", bufs=1) for t in range(2)]
    proj_stack.close()
    acts = ctx.enter_context(tc.tile_pool(name="tail", bufs=1))
    ho = []
    with tc.tile_pool(name="sc_ps", bufs=2, space="PSUM") as sc_ps, \
         tc.tile_pool(name="o_ps", bufs=2, space="PSUM") as o_ps, \
         tc.tile_pool(name="expp", bufs=8) as expp:
        for t in range(2):
            for h in range(4 * t, 4 * t + 4):
                ti, base = divmod(h, 4)
                eT = []
                for j in range(8):
                    L = S - j * 128
                    ps = sc_ps.tile([128, 1024], F32, tag="sc", name="ps")
                    p0 = 0
                    while p0 < L:
                        pl = min(512, L - p0)
                        if L - p0 - pl == 128:
                            pl = 384  # keep the tail piece >= 256 where possible
                        nc.tensor.matmul(
                            ps[:, p0:p0 + pl],
                            (k_head(h)[:, j * 128:(j + 1) * 128]),
                            (q_head(h)[:, j * 128 + p0:j * 128 + p0 + pl]),
                            start=True, stop=True)
                        p0 += pl
                    e = expp.tile([128, 1024], FR, tag="expT", name="e")
                    nc.scalar.activation(e[:, 0:L], ps[:, 0:L], ACTF.Exp, scale=INV_SQRT_DK)
                    nc.vector.tensor_tensor(e[:, 0:128], e[:, 0:128], causal[:], op=ALU.mult)
                    eT.append(e)
                # PV with fused denominator row
                ops = o_ps.tile([33, S], F32, tag="o", name="ops")
                for c2 in range(2):
                    cs, ce = c2 * 512, (c2 + 1) * 512
                    jmax = min(4 * c2 + 3, 7)
                    for j in range(jmax + 1):
                        qs = max(cs, j * 128)
                        nc.tensor.matmul(
                            ops[:, qs:ce],
                            (vt[j][:, h * 33:(h + 1) * 33]),
                            (eT[j][:, qs - j * 128:ce - j * 128]),
                            start=(j == 0), stop=(j == jmax))
                ohb = heads.tile([33, S], F32, name="ohb", tag="ohb", bufs=3)
                nc.scalar.activation(ohb[:], ops[:], ACTF.Copy)
                nc.sync.dma_start(out=attn_out[ti][base * 32:(base + 1) * 32, :], in_=ohb[0:32, :])
                nc.sync.dma_start(out=sums4[ti][base:base + 1, :], in_=ohb[32:33, :])

            # ---- group-t normalization + ELU(+1) ----
            recip = acts.tile([4, S], F32, name="recip", tag="recip", bufs=2)
            scr = acts.tile([4, S], F32, name="rscr", tag="rscr", bufs=2)
            nc.vector.reciprocal_approx_accurate(recip[:], sums4[t][:], scr[:])
            recip_r = acts.tile([4, S], FR, name="recip_r", tag="recip_r", bufs=2)
            nc.vector.tensor_copy(recip_r[:], recip[:])
            psr = o_ps.tile([128, S], F32, tag="o", name="psr")
            for (c0, c1) in HCH:
                nc.tensor.matmul(psr[:, c0:c1], (ind4[:]),
                                 (recip_r[:, c0:c1]), start=True, stop=True)
            onrm = acts.tile([128, S], F32, name="onrm", tag="onrm", bufs=2)
            nc.vector.tensor_tensor(onrm[:], attn_out[t][:], psr[:], op=ALU.mult)
            tr = acts.tile([128, S], F32, name="tr", tag="tr", bufs=2)
            te = acts.tile([128, S], F32, name="te", tag="te", bufs=2)
            hh = acts.tile([128, S], FR, name=f"ho{t}", tag=f"ho{t}", bufs=1)
            nc.vector.tensor_scalar(tr[:], onrm[:], 0.0, None, op0=ALU.max)
            nc.scalar.activation(te[:], onrm[:], ACTF.Exp)
            nc.vector.scalar_tensor_tensor(hh[:], te[:], 1.0, tr[:], op0=ALU.min, op1=ALU.add)
            ho.append(hh)

        # ---- output pointwise block + residual ----
        for ot in range(2):
            for (c0, c1) in HCH:
                ps = o_ps.tile([128, 512], F32, tag="o", name="pops")
                for it in range(2):
                    nc.tensor.matmul(
                        ps[:], (pwout_sb[it][:, ot * 128:(ot + 1) * 128]),
                        (ho[it][:, c0:c1]), start=(it == 0), stop=(it == 1))
                tr = acts.tile([128, 512], F32, name="ctr", tag="ctr", bufs=3)
                te = acts.tile([128, 512], F32, name="cte", tag="cte", bufs=3)
                u = acts.tile([128, 512], F32, name="fu", tag="fu", bufs=2)
                fin = acts.tile([128, 512], F32, name="fin", tag="fin", bufs=2)
                nc.vector.tensor_scalar(tr[:], ps[:], pwoutb[ot], 0.0, op0=ALU.add, op1=ALU.max)
                nc.scalar.activation(te[:], ps[:], ACTF.Exp, bias=pwoutb[ot])
                nc.vector.scalar_tensor_tensor(u[:], te[:], 1.0, tr[:], op0=ALU.min, op1=ALU.add)
                nc.vector.scalar_tensor_tensor(
                    fin[:].rearrange("p (r c) -> p r c", r=16, c=32),
                    u[:].rearrange("p (r c) -> p r c", r=16, c=32), -1.0,
                    res3[ot][:, c0 // 32:c1 // 32, :],
                    op0=ALU.add, op1=ALU.add)
                nc.sync.dma_start(out=out_d[ot * 128:(ot + 1) * 128, c0:c1], in_=fin[:])


_CACHED_NC = None


def _get_nc():
    global _CACHED_NC
    if _CACHED_NC is None:
        _CACHED_NC = _build_program()
    return _CACHED_NC


def _prep_host(inputs):
    """Host-side packing: shard input over batch, pre-transpose weights,
    fold the elu+1 corrections into effective biases."""
    f = np.float32
    rb_w_in = np.asarray(inputs["rb_w_in"], f)
    rb_w_out = np.asarray(inputs["rb_w_out"], f)
    wv = np.asarray(inputs["wv"], f)

    # [r, tap, i, o] -> packed [r, it, p(128), tap*o]
    c1t = rb_w_in.transpose(0, 3, 4, 2, 1).reshape(2, 6, C, C)
    c1w = np.ascontiguousarray(
        c1t.reshape(2, 6, 2, 128, C).transpose(0, 2, 3, 1, 4).reshape(2, 2, 128, 6 * C)
        .astype(ml_dtypes.bfloat16))
    c2t = rb_w_out.transpose(0, 3, 4, 2, 1).reshape(2, 6, C, 2 * C)
    c2w = np.ascontiguousarray(
        c2t.reshape(2, 6, 2, 128, 2 * C).transpose(0, 2, 3, 1, 4).reshape(2, 2, 128, 12 * C)
        .astype(ml_dtypes.bfloat16))
    b1e = inputs["rb_b_in"] - rb_w_in.sum((2, 3, 4))
    b2e = inputs["rb_b_out"] - rb_w_out.sum((2, 3, 4))
    common = {
        "c1w": c1w,
        "c2w": c2w,
        "b1e": np.ascontiguousarray(b1e, f),
        "b2a": np.ascontiguousarray(b2e[:, :C], f),
        "b2gh": np.ascontiguousarray(0.5 * b2e[:, C:], f),
        "apw": np.ascontiguousarray(
            np.stack([np.asarray(inputs["pw_in_w"], f).T,
                      np.asarray(inputs["wq"], f).T,
                      np.asarray(inputs["wk"], f).T,
                      wv.T,
                      np.asarray(inputs["pw_out_w"], f).T])  # [5, c, o]
            .reshape(5, 2, 128, C).transpose(1, 2, 0, 3).reshape(2, 128, 5 * C)),
        "pwinb": np.asarray(inputs["pw_in_b"] - np.asarray(inputs["pw_in_w"], f).sum(1), f),
        "bqe": np.asarray(inputs["bq"] - np.asarray(inputs["wq"], f).sum(1), f),
        "bke": np.asarray(inputs["bk"] - np.asarray(inputs["wk"], f).sum(1), f),
        "s2row": np.ascontiguousarray((wv.sum(1) - np.asarray(inputs["bv"], f))[None, :]),
        "pwoutb": np.asarray(
            inputs["pw_out_b"] - np.asarray(inputs["pw_out_w"], f).sum(1), f),
    }
    common = {k: (v if v.dtype == ml_dtypes.bfloat16 else np.ascontiguousarray(v, f)) for k, v in common.items()}
    inp = np.asarray(inputs["input"], f)
    in_maps = []
    for c in range(8):
        m = dict(common)
        m["inp"] = np.ascontiguousarray(inp[c].reshape(C, S))
        in_maps.append(m)
    return in_maps


def kernel(**inputs) -> np.ndarray:
    nc = _get_nc()
    in_maps = _prep_host(inputs)
    res = run_bass_kernel_spmd(nc, in_maps, list(range(8)))
    out = np.stack([res.results[c]["out"].reshape(C, 32, 32) for c in range(8)])
    return out.astype(np.float32)


def run_traced(inputs):
    """For test.py: run with NTFF profiling, returns (output, exec_time_ns)."""
    import types
    import trn_agent_boot.trn_boot as tb
    hook = tb._ntff_profile_via_ctypes("/opt/axon/libaxon_pjrt.so")
    mod = types.ModuleType("antenv.axon_hooks")
    mod.get_axon_ntff_profile_hook = lambda: hook
    import antenv
    sys.modules["antenv.axon_hooks"] = mod
    antenv.axon_hooks = mod

    nc = _get_nc()
    in_maps = _prep_host(inputs)
    res = run_bass_kernel_spmd(nc, in_maps, list(range(8)), trace=True)
    out = np.stack([res.results[c]["out"].reshape(C, 32, 32) for c in range(8)])
    return out.astype(np.float32), res.exec_time_ns


# revision 25
# speedup vs baseline: 1.0647x; 1.0563x over previous
"""Trainium2 Bass kernel for nn_PixelBlock (causal-conv GLU backbone + causal
self-attention block). Data-parallel over batch: 8 images -> 8 NeuronCores.

Layout notes (per core, B=1):
- Activations are channel-major [C, S] with C on partitions (256 -> 2 tiles).
- For the 2x3 causal convs, spatial is kept in a padded layout: 33 rows
  (1 top-pad) x 34 cols (2 left-pad) = 1122 flat positions. Each conv tap is
  then a pure flat shift, so conv = 6 accumulated matmuls per K-tile.
- ELU(x) is computed as max(x,0) + min(exp(x),1) - 1. We carry h' = elu+1
  (so padding cells hold exactly 1.0) and fold the -1 into the next layer's
  bias via  b_eff = b - W @ ones  (computed on host).
- Sigmoid is computed via tanh (same ACT table set as exp -> no table swaps).
- Attention: scores are built transposed (sT[k,q] = k_h^T q_h), exp'd with
  the 1/sqrt(dk) scale folded into the ACT scale, diagonal 128-blocks masked
  with a 0/1 lower-triangle mask. PV uses V^T (computed directly by a
  transposed projection) augmented with a ones-column so the softmax
  denominators fall out of the same matmul. Output is normalized by a
  reciprocal broadcast through a tiny K=8 indicator matmul.
"""

import sys
import numpy as np
import ml_dtypes

sys.path.insert(0, "/opt/trn_rl_repo")

from contextlib import ExitStack

import concourse.bass as bass
import concourse.tile as tile
from concourse import bacc, mybir
from concourse.bass_utils import run_bass_kernel_spmd

F32 = mybir.dt.float32
FR = mybir.dt.float32r
BF = mybir.dt.bfloat16
ALU = mybir.AluOpType
ACTF = mybir.ActivationFunctionType

C = 256
S = 1024
NH = 8
DK = 32
PW = 34          # padded row width (2 left pad + 32)
NROW = 33        # 1 top pad + 32 rows
PTOT = NROW * PW  # 1122
P0 = 36          # first valid output position (row 1, col 2)
CHUNK = 362      # conv output chunk (3 * 362 = 1086 = PTOT - P0)
CHUNKS = [(P0 + i * CHUNK, P0 + (i + 1) * CHUNK) for i in range(3)]
# tap shifts for (di, dj) in row-major order, shift = (di-1)*34 + (dj-2)
SHIFTS = [-36, -35, -34, -2, -1, 0]
INV_SQRT_DK = 1.0 / np.sqrt(32.0)


def _build_program():
    nc = bacc.Bacc("TRN2", target_bir_lowering=False, debug=False, num_devices=8)

    d = {}
    def din(name, shape, dt=F32):
        d[name] = nc.dram_tensor(name, list(shape), dt, kind="ExternalInput").ap()
    din("inp", (C, S))
    din("cw", (2, 2, 128, 18 * C), BF)    # [r, it, p, c1-taps*o | c2-taps*o]
    din("apw", (2, 128, 5 * C), FR)       # [it, p, {pwin,wq,wk,wv,pwout}*o]
    din("bcols", (2, 128, 10))            # all per-partition bias columns
    din("s2row", (1, C), FR)        # wv.sum(1) - bv
    out_d = nc.dram_tensor("out", [C, S], F32, kind="ExternalOutput").ap()

    with tile.TileContext(nc) as tc, ExitStack() as ctx:
        _kernel_body(ctx, tc, nc, d, out_d)

    nc.compile()
    return nc




def _kernel_body(ctx, tc, nc, d, out_d):
    consts = ctx.enter_context(tc.tile_pool(name="consts", bufs=1))
    wq_pool = ctx.enter_context(tc.tile_pool(name="wq", bufs=1))

    # ---- constants (gpsimd-built masks are emitted in the attention
    # section: gpsimd takes ~26us to boot and must stay off the critical
    # path of the backbone's DVE stream) ----
    ones_f = consts.tile([1, 128], F32)
    nc.vector.memset(ones_f[:], 1.0)
    ones_row = consts.tile([1, 128], FR)
    nc.vector.tensor_copy(ones_row[:], ones_f[:])
    ones8 = consts.tile([128, 8], F32)
    nc.vector.memset(ones8[:], 1.0)
    ones64 = consts.tile([128, 64], F32)
    nc.vector.memset(ones64[:], 1.0)

    def build_masks():
        causal = consts.tile([128, 128], F32, name="causal")   # 1 where k <= q
        nc.gpsimd.memset(causal[:], 1.0)
        nc.gpsimd.affine_select(
            out=causal[:], in_=causal[:], compare_op=ALU.is_ge, fill=0.0,
            base=0, channel_multiplier=-1, pattern=[[1, 128]])
        ind4_f = consts.tile([4, 128], F32, name="ind4_f")     # 1 where m//32 == k
        nc.gpsimd.memset(ind4_f[:], 1.0)
        nc.gpsimd.affine_select(
            out=ind4_f[:], in_=ind4_f[:], compare_op=ALU.is_ge, fill=0.0,
            base=0, channel_multiplier=-32, pattern=[[1, 128]])
        nc.gpsimd.affine_select(
            out=ind4_f[:], in_=ind4_f[:], compare_op=ALU.is_ge, fill=0.0,
            base=31, channel_multiplier=32, pattern=[[-1, 128]])
        ind4 = consts.tile([4, 128], FR, name="ind4")
        nc.vector.tensor_copy(ind4[:], ind4_f[:])
        return causal, ind4


    # ---- attention / pointwise weights (packed, persistent) ----
    def load_apw():
        groups = [[], [], [], [], []]
        for t in range(2):
            w = wq_pool.tile([128, 5 * C], FR, tag=f"apw{t}", name=f"apw{t}")
            nc.sync.dma_start(out=w[:], in_=d["apw"][t])
            for g in range(5):
                groups[g].append(w[:, g * C:(g + 1) * C])
        return groups

    # ---- input -> padded layout ----
    def pad3(t2d):
        return t2d.rearrange("p (r c) -> p r c", r=NROW, c=PW)

    bb_stack = ExitStack()
    bb = bb_stack.enter_context(tc.tile_pool(name="bb", bufs=1))
    x_cur = []
    for t in range(2):
        x0 = bb.tile([128, PTOT], F32, name=f"x0_{t}", tag=f"xp{t}", bufs=2)
        nc.vector.memset(x0[:, 0:P0], 0.0)
        nc.vector.memset(pad3(x0)[:, 2:33, 0:2], 0.0)
        for hf in range(2):
            stg = bb.tile([128, 512], F32, name=f"stg{t}{hf}", tag="stg", bufs=4)
            nc.sync.dma_start(out=stg[:], in_=d["inp"][t * 128:(t + 1) * 128, hf * 512:(hf + 1) * 512])
            nc.vector.tensor_copy(
                pad3(x0)[:, 1 + 16 * hf:17 + 16 * hf, 2:34],
                stg[:].rearrange("p (r c) -> p r c", r=16, c=32))
        x_cur.append(x0)

    # ---- biases: one packed [128, 10] tile per channel half ----
    bct = []
    for t in range(2):
        bc = consts.tile([128, 10], F32, name=f"bcols{t}")
        nc.sync.dma_start(out=bc[:], in_=d["bcols"][t])
        bct.append(bc)
    def bcol(i):
        return [bct[0][:, i:i + 1], bct[1][:, i:i + 1]]
    b1 = [bcol(0), bcol(1)]
    b2a = [bcol(2), bcol(3)]
    b2gh = [bcol(4), bcol(5)]
    pwinb = bcol(6)
    bqe = bcol(7)
    bke = bcol(8)
    pwoutb = bcol(9)

    # =========================== backbone ===========================
    with tc.tile_pool(name="convw", bufs=1) as convw, \
         tc.tile_pool(name="c1ps", bufs=4, space="PSUM") as c1ps, \
         tc.tile_pool(name="c2ps", bufs=4, space="PSUM") as c2ps:
        # all conv weights: 4 big DMAs (one per repeat x K-tile), issued from
        # the scalar sequencer (ACT is idle at startup; SP is the bottleneck)
        cwsb = {}
        for r_ in range(2):
            for it in range(2):
                w = convw.tile([128, 18 * C], BF, tag="cw", bufs=4, name="w")
                nc.scalar.dma_start(out=w[:], in_=d["cw"][r_, it])
                cwsb[(r_, it)] = w
        for r in range(2):
            c1sb = [cwsb[(r, it)][:, 0:6 * C] for it in range(2)]
            c2sb = [cwsb[(r, it)][:, 6 * C:18 * C] for it in range(2)]

            # h1' = ELU(x)+1 over the full padded tile (pads stay exactly 1)
            h1 = []
            for t in range(2):
                h = bb.tile([128, PTOT], BF, name=f"h1_{t}", tag=f"h1_{t}", bufs=1)
                for (r0_, r1_) in ((0, 578), (578, PTOT)):
                    tr = bb.tile([128, 578], F32, name="tr", tag="btr", bufs=3)
                    te = bb.tile([128, 578], F32, name="te", tag="bte", bufs=3)
                    w = r1_ - r0_
                    nc.vector.tensor_scalar(tr[:, 0:w], x_cur[t][:, r0_:r1_], 0.0, None, op0=ALU.max)
                    nc.scalar.activation(te[:, 0:w], x_cur[t][:, r0_:r1_], ACTF.Exp)
                    nc.vector.scalar_tensor_tensor(
                        h[:, r0_:r1_], te[:, 0:w], 1.0, tr[:, 0:w], op0=ALU.min, op1=ALU.add)
                h1.append(h)

            # conv1 -> h2' = ELU(y1 + b1_eff)+1
            h2 = []
            for t in range(2):
                h = bb.tile([128, PTOT], BF, name=f"h2_{t}", tag=f"h2_{t}", bufs=1)
                h2.append(h)
            for (s0, e0) in CHUNKS:
                for ot in range(2):
                    ps = c1ps.tile([128, CHUNK], F32, tag="c1")
                    n = 0
                    for tap in range(6):
                        for it in range(2):
                            nc.tensor.matmul(
                                ps[:],
                                (c1sb[it][:, tap * 256 + ot * 128:tap * 256 + (ot + 1) * 128]),
                                (h1[it][:, s0 + SHIFTS[tap]:e0 + SHIFTS[tap]]),
                                start=(n == 0), stop=(n == 11))
                            n += 1
                    tr = bb.tile([128, CHUNK], F32, name="ctr", tag="bctr", bufs=3)
                    te = bb.tile([128, CHUNK], F32, name="cte", tag="bcte", bufs=3)
                    nc.vector.tensor_scalar(tr[:], ps[:], b1[r][ot], 0.0, op0=ALU.add, op1=ALU.max)
                    nc.scalar.activation(te[:], ps[:], ACTF.Exp, bias=b1[r][ot])
                    nc.vector.scalar_tensor_tensor(
                        h2[ot][:, s0:e0], te[:], 1.0, tr[:], op0=ALU.min, op1=ALU.add)

            # pads of h2' must be exactly 1 -- written AFTER the chunk writes
            # (the chunks cover the in-row pad columns with garbage)
            for t in range(2):
                nc.vector.tensor_copy(h2[t][:, 0:P0], ones64[:, 0:P0])
                nc.vector.tensor_copy(
                    pad3(h2[t])[:, 2:33, 0:2],
                    ones64[:, 0:62].rearrange("p (r c) -> p r c", r=31, c=2))

            # conv2 -> GLU -> x_new. On the last repeat the result (and its
            # elu+1) live in the persistent pool: attention reads them via
            # strided 3D views, so no compacting copies are needed.
            x_new = []
            xe_pad = []
            for t in range(2):
                if r == 0:
                    xn = bb.tile([128, PTOT], F32, name=f"xn{t}", tag=f"xp{t}", bufs=2)
                else:
                    xn = wq_pool.tile([128, PTOT], F32, name=f"xfin{t}", tag=f"xfin{t}")
                    xep = wq_pool.tile([128, PTOT], FR, name=f"xep{t}", tag=f"xep{t}")
                    xe_pad.append(xep)
                x_new.append(xn)
            for (s0, e0) in CHUNKS:
                pss = []
                for ot in range(4):
                    ps = c2ps.tile([128, CHUNK], F32, tag="c2")
                    n = 0
                    for tap in range(6):
                        for it in range(2):
                            nc.tensor.matmul(
                                ps[:],
                                (c2sb[it][:, tap * 512 + ot * 128:tap * 512 + (ot + 1) * 128]),
                                (h2[it][:, s0 + SHIFTS[tap]:e0 + SHIFTS[tap]]),
                                start=(n == 0), stop=(n == 11))
                            n += 1
                    pss.append(ps)
                for t in range(2):
                    # sigmoid(g) = 0.5*(1 + tanh(g/2)); x += a * sigmoid(g)
                    th = bb.tile([128, CHUNK], F32, name="th", tag="th", bufs=3)
                    nc.scalar.activation(th[:], pss[2 + t][:], ACTF.Tanh,
                                         bias=b2gh[r][t], scale=0.5)
                    ah = bb.tile([128, CHUNK], F32, name="ah", tag="ah", bufs=3)
                    nc.vector.tensor_scalar(ah[:], pss[t][:], b2a[r][t], 0.5,
                                            op0=ALU.add, op1=ALU.mult)
                    gl = bb.tile([128, CHUNK], F32, name="gl", tag="gl", bufs=3)
                    nc.vector.scalar_tensor_tensor(gl[:], th[:], 1.0, ah[:],
                                                   op0=ALU.add, op1=ALU.mult)
                    nc.vector.tensor_tensor(x_new[t][:, s0:e0], x_cur[t][:, s0:e0],
                                            gl[:], op=ALU.add)
                    if r == 1:
                        # fused: xe' = ELU(x_new)+1 per chunk (overlaps conv2)
                        tr = bb.tile([128, CHUNK], F32, name="xtr", tag="bctr", bufs=3)
                        te = bb.tile([128, CHUNK], F32, name="xte", tag="bcte", bufs=3)
                        nc.vector.tensor_scalar(tr[:], x_new[t][:, s0:e0], 0.0, None, op0=ALU.max)
                        nc.scalar.activation(te[:], x_new[t][:, s0:e0], ACTF.Exp)
                        nc.vector.scalar_tensor_tensor(
                            xe_pad[t][:, s0:e0], te[:], 1.0, tr[:], op0=ALU.min, op1=ALU.add)
            if r == 0:
                for t in range(2):
                    nc.vector.memset(x_new[t][:, 0:P0], 0.0)
                    nc.vector.memset(pad3(x_new[t])[:, 2:33, 0:2], 0.0)
            x_cur = x_new

    # =========================== attention ===========================
    bb_stack.close()
    # interior 3D views of the persistent padded tiles
    res3 = [pad3(x_cur[t])[:, 1:33, 2:34] for t in range(2)]       # residual
    xe3 = [pad3(xe_pad[t])[:, 1:33, 2:34] for t in range(2)]       # elu(res)+1

    def xe_cols(t, c0, c1):
        assert c0 % 32 == 0 and c1 % 32 == 0
        return xe3[t][:, c0 // 32:c1 // 32, :]

    pwin_sb, wq_sb, wk_sb, wv_sb, pwout_sb = load_apw()
    causal, ind4 = build_masks()
    heads = ctx.enter_context(tc.tile_pool(name="heads", bufs=1))
    proj_stack = ExitStack()
    proj = proj_stack.enter_context(tc.tile_pool(name="proj", bufs=1))

    HCH = [(0, 512), (512, 1024)]
    with tc.tile_pool(name="prep_ps", bufs=4, space="PSUM") as prep_ps, \
         tc.tile_pool(name="vt_ps", bufs=2, space="PSUM") as vt_ps:
        # pw_in -> x' (elu+1 of pointwise output)
        xp = [proj.tile([128, S], FR, name=f"xpa{t}", tag=f"xpa{t}", bufs=1) for t in range(2)]
        for ot in range(2):
            for (c0, c1) in HCH:
                ps = prep_ps.tile([128, 512], F32, tag="pp")
                for it in range(2):
                    nc.tensor.matmul(
                        ps[:], (pwin_sb[it][:, ot * 128:(ot + 1) * 128]),
                        xe_cols(it, c0, c1), start=(it == 0), stop=(it == 1))
                tr = proj.tile([128, 512], F32, name="ctr", tag="ctr", bufs=3)
                te = proj.tile([128, 512], F32, name="cte", tag="cte", bufs=3)
                nc.vector.tensor_scalar(tr[:], ps[:], pwinb[ot], 0.0, op0=ALU.add, op1=ALU.max)
                nc.scalar.activation(te[:], ps[:], ACTF.Exp, bias=pwinb[ot])
                nc.vector.scalar_tensor_tensor(
                    xp[ot][:, c0:c1], te[:], 1.0, tr[:], op0=ALU.min, op1=ALU.add)

        # q, k projections (biased, scale folded into score exp later)
        q_sb = [heads.tile([128, S], FR, name=f"q{t}", tag=f"q{t}", bufs=1) for t in range(2)]
        k_sb = [heads.tile([128, S], FR, name=f"k{t}", tag=f"k{t}", bufs=1) for t in range(2)]
        for (wsb, osb, bias) in ((wq_sb, q_sb, bqe), (wk_sb, k_sb, bke)):
            for ot in range(2):
                for (c0, c1) in HCH:
                    ps = prep_ps.tile([128, 512], F32, tag="pp")
                    for it in range(2):
                        nc.tensor.matmul(
                            ps[:], (wsb[it][:, ot * 128:(ot + 1) * 128]),
                            (xp[it][:, c0:c1]), start=(it == 0), stop=(it == 1))
                    nc.vector.tensor_scalar(osb[ot][:, c0:c1], ps[:], bias[ot], None, op0=ALU.add)

        # matmul operands need partition base in {0,32,64}; heads 3 and 7 sit
        # at base 96, so relocate those two into one extra tile pair via DMA.
        q37 = heads.tile([64, S], FR, name="q37", tag="q37", bufs=1)
        k37 = heads.tile([64, S], FR, name="k37", tag="k37", bufs=1)
        for ti in range(2):
            nc.sync.dma_start(out=q37[ti * 32:(ti + 1) * 32, :], in_=q_sb[ti][96:128, :])
            nc.sync.dma_start(out=k37[ti * 32:(ti + 1) * 32, :], in_=k_sb[ti][96:128, :])

        def q_head(h):
            ti, b = divmod(h, 4)
            if b < 3:
                return q_sb[ti][b * 32:(b + 1) * 32, :]
            return q37[ti * 32:(ti + 1) * 32, :]

        def k_head(h):
            ti, b = divmod(h, 4)
            if b < 3:
                return k_sb[ti][b * 32:(b + 1) * 32, :]
            return k37[ti * 32:(ti + 1) * 32, :]

        # S2 broadcast tile: (wv.sum(1) - bv) replicated to 128 partitions
        s2row = proj.tile([1, 256], FR, name="s2row", tag="s2row", bufs=1)
        nc.sync.dma_start(out=s2row[:], in_=d["s2row"][:, :])
        ps_s2 = vt_ps.tile([128, 256], F32, tag="s2")
        nc.tensor.matmul(ps_s2[:], (ones_row[:]), (s2row[:]), start=True, stop=True)
        s2_sb = proj.tile([128, 256], F32, name="s2sb", tag="s2sb", bufs=1)
        nc.scalar.activation(s2_sb[:], ps_s2[:], ACTF.Copy)

        # V^T projection (x' as stationary), head-interleaved with ones cols
        vt = []
        for st in range(8):
            ps = vt_ps.tile([128, 256], F32, tag="vt")
            for it in range(2):
                nc.tensor.matmul(
                    ps[:], (xp[it][:, st * 128:(st + 1) * 128]),
                    (wv_sb[it][:]), start=(it == 0), stop=(it == 1))
            v = heads.tile([128, NH * 33], FR, name="vt", tag="vt", bufs=8)
            v3 = v.rearrange("p (h e) -> p h e", h=NH, e=33)
            nc.vector.tensor_tensor(
                v3[:, :, 0:32],
                ps[:].rearrange("p (h e) -> p h e", h=NH, e=32),
                s2_sb[:].rearrange("p (h e) -> p h e", h=NH, e=32),
                op=ALU.subtract)
            nc.vector.tensor_copy(v3[:, :, 32:33],
                                  ones8[:].rearrange("p (h e) -> p h e", e=1))
            vt.append(v)

    # heads + per-group normalization tail, sharing one PSUM budget:
    # tags: "sc" (scores / pw_out) and "o" (PV out / recip broadcast), 2 bufs
    # each of 2 banks -> 8 banks total. The group-t tail is emitted right
    # after its 4 heads so it overlaps the other group's compute.
    attn_out = [wq_pool.tile([128, S], F32, name=f"ao{t}", tag=f"ao{t}", bufs=1) for t in range(2)]
    sums4 = [wq_pool.tile([4, S], F32, name=f"sums{t}", tag=f"sums{t}", bufs=1) for t in range(2)]
    proj_stack.close()
    acts = ctx.enter_context(tc.tile_pool(name="tail", bufs=1))
    ho = []
    with tc.tile_pool(name="sc_ps", bufs=2, space="PSUM") as sc_ps, \
         tc.tile_pool(name="o_ps", bufs=2, space="PSUM") as o_ps, \
         tc.tile_pool(name="expp", bufs=8) as expp:
        for t in range(2):
            for h in range(4 * t, 4 * t + 4):
                ti, base = divmod(h, 4)
                eT = []
                for j in range(8):
                    L = S - j * 128
                    ps = sc_ps.tile([128, 1024], F32, tag="sc", name="ps")
                    p0 = 0
                    while p0 < L:
                        pl = min(512, L - p0)
                        if L - p0 - pl == 128:
                            pl = 384  # keep the tail piece >= 256 where possible
                        nc.tensor.matmul(
                            ps[:, p0:p0 + pl],
                            (k_head(h)[:, j * 128:(j + 1) * 128]),
                            (q_head(h)[:, j * 128 + p0:j * 128 + p0 + pl]),
                            start=True, stop=True)
                        p0 += pl
                    e = expp.tile([128, 1024], FR, tag="expT", name="e")
                    nc.scalar.activation(e[:, 0:L], ps[:, 0:L], ACTF.Exp, scale=INV_SQRT_DK)
                    nc.vector.tensor_tensor(e[:, 0:128], e[:, 0:128], causal[:], op=ALU.mult)
                    eT.append(e)
                # PV with fused denominator row
                ops = o_ps.tile([33, S], F32, tag="o", name="ops")
                for c2 in range(2):
                    cs, ce = c2 * 512, (c2 + 1) * 512
                    jmax = min(4 * c2 + 3, 7)
                    for j in range(jmax + 1):
                        qs = max(cs, j * 128)
                        nc.tensor.matmul(
                            ops[:, qs:ce],
                            (vt[j][:, h * 33:(h + 1) * 33]),
                            (eT[j][:, qs - j * 128:ce - j * 128]),
                            start=(j == 0), stop=(j == jmax))
                ohb = heads.tile([33, S], F32, name="ohb", tag="ohb", bufs=3)
                nc.scalar.activation(ohb[:], ops[:], ACTF.Copy)
                nc.sync.dma_start(out=attn_out[ti][base * 32:(base + 1) * 32, :], in_=ohb[0:32, :])
                nc.sync.dma_start(out=sums4[ti][base:base + 1, :], in_=ohb[32:33, :])

            # ---- group-t normalization + ELU(+1) ----
            recip = acts.tile([4, S], F32, name="recip", tag="recip", bufs=2)
            scr = acts.tile([4, S], F32, name="rscr", tag="rscr", bufs=2)
            nc.vector.reciprocal_approx_accurate(recip[:], sums4[t][:], scr[:])
            recip_r = acts.tile([4, S], FR, name="recip_r", tag="recip_r", bufs=2)
            nc.vector.tensor_copy(recip_r[:], recip[:])
            psr = o_ps.tile([128, S], F32, tag="o", name="psr")
            for (c0, c1) in HCH:
                nc.tensor.matmul(psr[:, c0:c1], (ind4[:]),
                                 (recip_r[:, c0:c1]), start=True, stop=True)
            onrm = acts.tile([128, S], F32, name="onrm", tag="onrm", bufs=2)
            nc.vector.tensor_tensor(onrm[:], attn_out[t][:], psr[:], op=ALU.mult)
            tr = acts.tile([128, S], F32, name="tr", tag="tr", bufs=2)
            te = acts.tile([128, S], F32, name="te", tag="te", bufs=2)
            hh = acts.tile([128, S], FR, name=f"ho{t}", tag=f"ho{t}", bufs=1)
            nc.vector.tensor_scalar(tr[:], onrm[:], 0.0, None, op0=ALU.max)
            nc.scalar.activation(te[:], onrm[:], ACTF.Exp)
            nc.vector.scalar_tensor_tensor(hh[:], te[:], 1.0, tr[:], op0=ALU.min, op1=ALU.add)
            ho.append(hh)

        # ---- output pointwise block + residual ----
        for ot in range(2):
            for (c0, c1) in HCH:
                ps = o_ps.tile([128, 512], F32, tag="o", name="pops")
                for it in range(2):
                    nc.tensor.matmul(
                        ps[:], (pwout_sb[it][:, ot * 128:(ot + 1) * 128]),
                        (ho[it][:, c0:c1]), start=(it == 0), stop=(it == 1))
                tr = acts.tile([128, 512], F32, name="ctr", tag="ctr", bufs=3)
                te = acts.tile([128, 512], F32, name="cte", tag="cte", bufs=3)
                u = acts.tile([128, 512], F32, name="fu", tag="fu", bufs=2)
                fin = acts.tile([128, 512], F32, name="fin", tag="fin", bufs=2)
                nc.vector.tensor_scalar(tr[:], ps[:], pwoutb[ot], 0.0, op0=ALU.add, op1=ALU.max)
                nc.scalar.activation(te[:], ps[:], ACTF.Exp, bias=pwoutb[ot])
                nc.vector.scalar_tensor_tensor(u[:], te[:], 1.0, tr[:], op0=ALU.min, op1=ALU.add)
                nc.vector.scalar_tensor_tensor(
                    fin[:].rearrange("p (r c) -> p r c", r=16, c=32),
                    u[:].rearrange("p (r c) -> p r c", r=16, c=32), -1.0,
                    res3[ot][:, c0 // 32:c1 // 32, :],
                    op0=ALU.add, op1=ALU.add)
                nc.sync.dma_start(out=out_d[ot * 128:(ot + 1) * 128, c0:c1], in_=fin[:])


_CACHED_NC = None


def _get_nc():
    global _CACHED_NC
    if _CACHED_NC is None:
        _CACHED_NC = _build_program()
    return _CACHED_NC


def _prep_host(inputs):
    """Host-side packing: shard input over batch, pre-transpose weights,
    fold the elu+1 corrections into effective biases."""
    f = np.float32
    rb_w_in = np.asarray(inputs["rb_w_in"], f)
    rb_w_out = np.asarray(inputs["rb_w_out"], f)
    wv = np.asarray(inputs["wv"], f)

    # [r, tap, i, o] -> packed [r, it, p(128), tap*o], c1 and c2 merged
    c1t = rb_w_in.transpose(0, 3, 4, 2, 1).reshape(2, 6, C, C)
    c1w = c1t.reshape(2, 6, 2, 128, C).transpose(0, 2, 3, 1, 4).reshape(2, 2, 128, 6 * C)
    c2t = rb_w_out.transpose(0, 3, 4, 2, 1).reshape(2, 6, C, 2 * C)
    c2w = c2t.reshape(2, 6, 2, 128, 2 * C).transpose(0, 2, 3, 1, 4).reshape(2, 2, 128, 12 * C)
    cw = np.ascontiguousarray(
        np.concatenate([c1w, c2w], axis=3).astype(ml_dtypes.bfloat16))
    b1e = inputs["rb_b_in"] - rb_w_in.sum((2, 3, 4))
    b2e = inputs["rb_b_out"] - rb_w_out.sum((2, 3, 4))
    bcols = np.stack([b1e[0], b1e[1], b2e[0, :C], b2e[1, :C],
                      0.5 * b2e[0, C:], 0.5 * b2e[1, C:],
                      inputs["pw_in_b"] - np.asarray(inputs["pw_in_w"], f).sum(1),
                      inputs["bq"] - np.asarray(inputs["wq"], f).sum(1),
                      inputs["bk"] - np.asarray(inputs["wk"], f).sum(1),
                      inputs["pw_out_b"] - np.asarray(inputs["pw_out_w"], f).sum(1)],
                     axis=1)  # [256, 10]
    common = {
        "cw": cw,
        "bcols": np.ascontiguousarray(bcols.reshape(2, 128, 10), f),
        "apw": np.ascontiguousarray(
            np.stack([np.asarray(inputs["pw_in_w"], f).T,
                      np.asarray(inputs["wq"], f).T,
                      np.asarray(inputs["wk"], f).T,
                      wv.T,
                      np.asarray(inputs["pw_out_w"], f).T])  # [5, c, o]
            .reshape(5, 2, 128, C).transpose(1, 2, 0, 3).reshape(2, 128, 5 * C)),
        "s2row": np.ascontiguousarray((wv.sum(1) - np.asarray(inputs["bv"], f))[None, :]),
    }
    common = {k: (v if v.dtype == ml_dtypes.bfloat16 else np.ascontiguousarray(v, f)) for k, v in common.items()}
    inp = np.asarray(inputs["input"], f)
    in_maps = []
    for c in range(8):
        m = dict(common)
        m["inp"] = np.ascontiguousarray(inp[c].reshape(C, S))
        in_maps.append(m)
    return in_maps


def kernel(**inputs) -> np.ndarray:
    nc = _get_nc()
    in_maps = _prep_host(inputs)
    res = run_bass_kernel_spmd(nc, in_maps, list(range(8)))
    out = np.stack([res.results[c]["out"].reshape(C, 32, 32) for c in range(8)])
    return out.astype(np.float32)


def run_traced(inputs):
    """For test.py: run with NTFF profiling, returns (output, exec_time_ns)."""
    import types
    import trn_agent_boot.trn_boot as tb
    hook = tb._ntff_profile_via_ctypes("/opt/axon/libaxon_pjrt.so")
    mod = types.ModuleType("antenv.axon_hooks")
    mod.get_axon_ntff_profile_hook = lambda: hook
    import antenv
    sys.modules["antenv.axon_hooks"] = mod
    antenv.axon_hooks = mod

    nc = _get_nc()
    in_maps = _prep_host(inputs)
    res = run_bass_kernel_spmd(nc, in_maps, list(range(8)), trace=True)
    out = np.stack([res.results[c]["out"].reshape(C, 32, 32) for c in range(8)])
    return out.astype(np.float32), res.exec_time_ns
